# revision 1
# baseline (speedup 1.0000x reference)
"""Gated multi-head attention, data-parallel over batch across 8 NeuronCores.

Shapes (hardcoded per problem spec):
  x:      [8, 1024, 768]
  qkv_w:  [768, 2304]
  qkv_b:  [2304]
  gate_w: [768, 768]
  proj_w: [768, 768]
B=8 batch elements map one-per-core across the 8 trn2 NeuronCores
(sharding_hint: data-parallel over batch). Weights are replicated.
Falls back to a pure-numpy implementation if the devices are unavailable.
"""

import numpy as np

B, N, C, H = 8, 1024, 768, 12
HD = C // H  # 64
SCALE = np.float32(1.0 / np.sqrt(HD))


def _softmax_np(a):
    m = a.max(axis=-1, keepdims=True)
    e = np.exp(a - m)
    return e / e.sum(axis=-1, keepdims=True)


def _numpy_path(x, qkv_w, qkv_b, gate_w, proj_w):
    out = np.empty((B, N, C), dtype=np.float32)
    for b in range(B):
        qkv = x[b] @ qkv_w + qkv_b  # [N, 3C]
        qkv = qkv.reshape(N, 3, H, HD).transpose(1, 2, 0, 3)  # [3,H,N,hd]
        q, k, v = qkv[0], qkv[1], qkv[2]
        attn = _softmax_np(np.einsum("hqd,hkd->hqk", q, k) * SCALE)
        o = np.einsum("hqk,hkd->hqd", attn, v)  # [H,N,hd]
        o = o.transpose(1, 0, 2).reshape(N, C) @ proj_w
        out[b] = o * (1.0 / (1.0 + np.exp(-(o @ gate_w))))
    return out


_PMAP_CACHE = {}


def _get_pmap():
    if "f" in _PMAP_CACHE:
        return _PMAP_CACHE["f"]
    import jax
    import jax.numpy as jnp

    devs = jax.devices()
    if len(devs) < 8:
        raise RuntimeError(f"need 8 devices, have {len(devs)}")

    def per_example(xb, qkv_w, qkv_b, gate_w, proj_w):
        qkv = xb @ qkv_w + qkv_b  # [N, 3C]
        qkv = qkv.reshape(N, 3, H, HD)
        qkv = jnp.transpose(qkv, (1, 2, 0, 3))  # [3,H,N,hd]
        q, k, v = qkv[0], qkv[1], qkv[2]
        attn = jnp.einsum("hqd,hkd->hqk", q, k) * SCALE
        attn = jax.nn.softmax(attn, axis=-1)
        o = jnp.einsum("hqk,hkd->hqd", attn, v)
        o = jnp.transpose(o, (1, 0, 2)).reshape(N, C) @ proj_w
        gate = jax.nn.sigmoid(o @ gate_w)
        return o * gate

    f = jax.pmap(
        per_example,
        in_axes=(0, None, None, None, None),
        devices=devs[:8],
    )
    _PMAP_CACHE["f"] = f
    return f


def kernel(**inputs):
    x = np.ascontiguousarray(np.asarray(inputs["x"], dtype=np.float32))
    qkv_w = np.ascontiguousarray(np.asarray(inputs["qkv_w"], dtype=np.float32))
    qkv_b = np.ascontiguousarray(np.asarray(inputs["qkv_b"], dtype=np.float32))
    gate_w = np.ascontiguousarray(np.asarray(inputs["gate_w"], dtype=np.float32))
    proj_w = np.ascontiguousarray(np.asarray(inputs["proj_w"], dtype=np.float32))
    try:
        f = _get_pmap()
        out = f(x, qkv_w, qkv_b, gate_w, proj_w)
        out = np.asarray(out, dtype=np.float32)
        if out.shape != (B, N, C) or not np.isfinite(out).all():
            raise RuntimeError("bad device output")
        return out
    except Exception:
        return _numpy_path(x, qkv_w, qkv_b, gate_w, proj_w)


# revision 2
# speedup vs baseline: 1.0060x; 1.0060x over previous
"""Gated multi-head attention, data-parallel over batch across 8 NeuronCores.

Shapes (hardcoded per problem spec):
  x:      [8, 1024, 768]
  qkv_w:  [768, 2304]
  qkv_b:  [2304]
  gate_w: [768, 768]
  proj_w: [768, 768]
B=8 batch elements map one-per-core across the 8 trn2 NeuronCores
(sharding_hint: data-parallel over batch). Weights are replicated.
Falls back to a pure-numpy implementation if the devices are unavailable.
"""

import numpy as np

B, N, C, H = 8, 1024, 768, 12
HD = C // H  # 64
SCALE = np.float32(1.0 / np.sqrt(HD))


def _softmax_np(a):
    m = a.max(axis=-1, keepdims=True)
    e = np.exp(a - m)
    return e / e.sum(axis=-1, keepdims=True)


def _numpy_path(x, qkv_w, qkv_b, gate_w, proj_w):
    out = np.empty((B, N, C), dtype=np.float32)
    for b in range(B):
        qkv = x[b] @ qkv_w + qkv_b  # [N, 3C]
        qkv = qkv.reshape(N, 3, H, HD).transpose(1, 2, 0, 3)  # [3,H,N,hd]
        q, k, v = qkv[0], qkv[1], qkv[2]
        attn = _softmax_np(np.einsum("hqd,hkd->hqk", q, k) * SCALE)
        o = np.einsum("hqk,hkd->hqd", attn, v)  # [H,N,hd]
        o = o.transpose(1, 0, 2).reshape(N, C) @ proj_w
        out[b] = o * (1.0 / (1.0 + np.exp(-(o @ gate_w))))
    return out


_PMAP_CACHE = {}


def _get_pmap():
    if "f" in _PMAP_CACHE:
        return _PMAP_CACHE["f"]
    import jax
    import jax.numpy as jnp

    try:
        jax.config.update("jax_compilation_cache_dir", "/tmp/jax_cc_cache")
        jax.config.update("jax_persistent_cache_min_compile_time_secs", 0.0)
    except Exception:
        pass

    devs = jax.devices()
    if len(devs) < 8:
        raise RuntimeError(f"need 8 devices, have {len(devs)}")

    def per_example(xb, qkv_w, qkv_b, gate_w, proj_w):
        qkv = xb @ qkv_w + qkv_b  # [N, 3C]
        qkv = qkv.reshape(N, 3, H, HD)
        qkv = jnp.transpose(qkv, (1, 2, 0, 3))  # [3,H,N,hd]
        q, k, v = qkv[0], qkv[1], qkv[2]
        attn = jnp.einsum("hqd,hkd->hqk", q, k) * SCALE
        attn = jax.nn.softmax(attn, axis=-1)
        o = jnp.einsum("hqk,hkd->hqd", attn, v)
        o = jnp.transpose(o, (1, 0, 2)).reshape(N, C) @ proj_w
        gate = jax.nn.sigmoid(o @ gate_w)
        return o * gate

    f = jax.pmap(
        per_example,
        in_axes=(0, None, None, None, None),
        devices=devs[:8],
    )
    _PMAP_CACHE["f"] = f
    return f


def kernel(**inputs):
    x = np.ascontiguousarray(np.asarray(inputs["x"], dtype=np.float32))
    qkv_w = np.ascontiguousarray(np.asarray(inputs["qkv_w"], dtype=np.float32))
    qkv_b = np.ascontiguousarray(np.asarray(inputs["qkv_b"], dtype=np.float32))
    gate_w = np.ascontiguousarray(np.asarray(inputs["gate_w"], dtype=np.float32))
    proj_w = np.ascontiguousarray(np.asarray(inputs["proj_w"], dtype=np.float32))
    try:
        f = _get_pmap()
        out = f(x, qkv_w, qkv_b, gate_w, proj_w)
        out = np.asarray(out, dtype=np.float32)
        if out.shape != (B, N, C) or not np.isfinite(out).all():
            raise RuntimeError("bad device output")
        return out
    except Exception:
        return _numpy_path(x, qkv_w, qkv_b, gate_w, proj_w)
